# revision 1
# baseline (speedup 1.0000x reference)
"""AreaWeightedDownsample (segment reduce) for Trainium2, 8 NeuronCores.

out[b, p, c] = sum_{n: parent[n]==p} omega[n] * x[b,n,c] / max(sum omega[n], eps)

Strategy:
  Host: fold omega/denom into per-row weights w'; sort rows by parent; shard
  (4 batches) x (2 sorted-row halves) across 8 cores. Pack rows into "slots"
  of 128 rows whose parents span < SLOT_W consecutive values.
  Device (SPMD, identical instruction stream on all cores): for each window of
  8 slots: DMA 1024 rows of x (f32->bf16 cast in flight); build the 8
  weighted-one-hot W tiles [128, SLOT_W] on VectorE from tiny per-slot tables
  (W[i, parent[i]-s_t] = w'[i] via iota==pl then *wv); 8 matmuls
  x_slot^T @ W -> psum[:, j*SLOT_W:(j+1)*SLOT_W]; drain psum -> SBUF (bf16)
  -> DRAM. This computes outT[c, parent] contributions per slot; the 4x row
  reduction happens in the PE array.
  Host: overlap-add the SLOT_W-wide slot slices at their parent offsets.
"""

import os
import sys

for _p in ("/opt/trn_rl_repo", "/opt/pypackages"):
    if _p not in sys.path:
        sys.path.insert(0, _p)

import numpy as np
import ml_dtypes

from concourse import bacc, mybir
import concourse.tile as tile
import concourse.bass_utils as _bass_utils
from concourse.bass_utils import run_bass_kernel_spmd

if os.environ.get("ENABLE_LDW_OPT") and not getattr(_bass_utils, "_ldw_patched", False):
    _bass_utils._ldw_patched = True
    _orig_rc = _bass_utils.run_command

    def _rc(argv, **kw):
        argv = [a.replace("--enable-ldw-opt=false", "--enable-ldw-opt=true")
                if isinstance(a, str) else a for a in argv]
        return _orig_rc(argv, **kw)

    _bass_utils.run_command = _rc

B = 4
N_IN = 163842
C = 128
N_OUT = 40962
EPS = 1e-8

P = 128          # rows per slot
SLOT_W = 32      # psum columns per slot (max parent span within a slot)
SPW = 16         # slots per window (SLOT_W*SPW*4B = one 2KB psum bank)
WIN_W = SLOT_W * SPW  # psum columns per window (<= 512, one bank)
OB = 8           # windows per output DMA
XB = 2           # windows per x DMA

LAST_IN_MAPS = None
LAST_NC = None

_NC_CACHE = {}


def build_nc(n_slots, repeat=1, no_w=False, no_drain=False, no_mm=False):
    """Build the SPMD device graph for n_slots slots (multiple of 8).

    repeat > 1 replays the whole program (for timing); output is idempotent.
    """
    assert n_slots % SPW == 0
    n_win = n_slots // SPW

    nc = bacc.Bacc(None, target_bir_lowering=False)
    x_d = nc.dram_tensor("xs", [P, n_slots, C], mybir.dt.float32,
                         kind="ExternalInput")
    iota_d = nc.dram_tensor("iota", [P, XB * WIN_W], mybir.dt.bfloat16,
                            kind="ExternalInput")
    pl_d = nc.dram_tensor("pl", [P, n_slots], mybir.dt.bfloat16,
                          kind="ExternalInput")
    wv_d = nc.dram_tensor("wv", [P, n_slots], mybir.dt.bfloat16,
                          kind="ExternalInput")
    o_d = nc.dram_tensor("out", [n_win, C, WIN_W], mybir.dt.bfloat16,
                         kind="ExternalOutput")

    with tile.TileContext(nc) as tc:
        with tc.tile_pool(name="cn", bufs=1) as cn, \
             tc.tile_pool(name="xfp", bufs=3) as xf_p, \
             tc.tile_pool(name="xp", bufs=3) as xp, \
             tc.tile_pool(name="wp", bufs=3) as wp, \
             tc.tile_pool(name="ip", bufs=3) as ip, \
             tc.tile_pool(name="sp", bufs=4) as sp, \
             tc.tile_pool(name="pp", bufs=6, space="PSUM") as pp:
            it = cn.tile([P, XB * WIN_W], mybir.dt.bfloat16)
            nc.sync.dma_start(out=it[:], in_=iota_d[:, :])
            plt = cn.tile([P, n_slots], mybir.dt.bfloat16)
            nc.sync.dma_start(out=plt[:], in_=pl_d[:, :])
            wvt = cn.tile([P, n_slots], mybir.dt.bfloat16)
            nc.sync.dma_start(out=wvt[:], in_=wv_d[:, :])
            wconst = None
            if no_w:
                wconst = cn.tile([P, XB * SPW, SLOT_W], mybir.dt.bfloat16)
                nc.vector.memset(wconst[:].rearrange("p t k -> p (t k)"), 0.5)

            xbatch = 0
            for _r in range(repeat):
                for g0 in range(0, n_win, OB):
                    gsz = min(OB, n_win - g0)
                    st = sp.tile([P, gsz, WIN_W], mybir.dt.bfloat16, tag="st")
                    for x0 in range(g0, g0 + gsz, XB):
                        xsz = min(XB, g0 + gsz - x0)
                        ns = xsz * SPW  # slots in this x-batch
                        xt = xp.tile([P, ns, C], mybir.dt.bfloat16, tag="xt")
                        if xbatch % 3 < 2:
                            # SWDGE path: cast f32->bf16 in the DMA (Q7 time)
                            nc.gpsimd.dma_start(
                                out=xt[:],
                                in_=x_d[:, x0 * SPW:(x0 + xsz) * SPW, :],
                            )
                        else:
                            # HWDGE f32 + ScalarE cast
                            xf = xf_p.tile([P, ns, C], mybir.dt.float32,
                                           tag="xf")
                            nc.sync.dma_start(
                                out=xf[:],
                                in_=x_d[:, x0 * SPW:(x0 + xsz) * SPW, :],
                            )
                            nc.scalar.copy(out=xt[:], in_=xf[:])
                        xbatch += 1
                        iseq = ip.tile([P, ns, SLOT_W], mybir.dt.bfloat16,
                                       tag="iseq")
                        if no_w:
                            wt = wconst
                        else:
                            nc.vector.tensor_tensor(
                            out=iseq[:],
                            in0=it[:, :ns * SLOT_W].rearrange(
                                "p (t k) -> p t k", k=SLOT_W),
                            in1=plt[:, x0 * SPW:(x0 + xsz) * SPW][:, :, None]
                                .to_broadcast([P, ns, SLOT_W]),
                            op=mybir.AluOpType.is_equal,
                            )
                            wt = wp.tile([P, ns, SLOT_W], mybir.dt.bfloat16,
                                         tag="wt")
                            nc.vector.tensor_tensor(
                            out=wt[:],
                            in0=iseq[:],
                            in1=wvt[:, x0 * SPW:(x0 + xsz) * SPW][:, :, None]
                                .to_broadcast([P, ns, SLOT_W]),
                            op=mybir.AluOpType.mult,
                            )
                        for dw in range(xsz):
                            w = x0 + dw
                            pt = pp.tile([P, WIN_W], mybir.dt.float32)
                            for j in range(SPW if not no_mm else 1):
                                cg = j % 4          # psum col-group
                                fs = j // 4         # psum free-slot
                                nc.tensor.matmul(
                                    out=pt[32 * cg:32 * cg + 32,
                                           C * fs:C * fs + C],
                                    lhsT=wt[:, dw * SPW + j, :],
                                    rhs=xt[:, dw * SPW + j, :],
                                    start=True, stop=True,
                                    tile_position=(0, 32 * cg),
                                )
                            if no_drain:
                                continue
                            if w % 2 == 0:
                                nc.vector.tensor_copy(
                                    out=st[:, w - g0, :], in_=pt[:])
                            else:
                                nc.scalar.copy(
                                    out=st[:, w - g0, :], in_=pt[:])
                    if not no_drain:
                        nc.sync.dma_start(
                            out=o_d[g0:g0 + gsz].rearrange("g p k -> p g k"),
                            in_=st[:],
                        )
    nc.compile()
    return nc


def _build_schedule(sps):
    """Greedy slot packing of sorted parents sps (ascending, int).

    Returns (starts, ends, s_arr): slot t covers sorted-row range
    [starts[t], ends[t]) (<=128 rows), parents in [s_arr[t], s_arr[t]+SLOT_W).
    """
    R = len(sps)
    starts, ends, s_arr = [], [], []
    i = 0
    while i < R:
        s = int(sps[i])
        j = min(i + P, R)
        j = min(j, int(np.searchsorted(sps, s + SLOT_W, side="left")))
        starts.append(i)
        ends.append(j)
        s_arr.append(s)
        i = j
    return np.array(starts), np.array(ends), np.array(s_arr)


def prep(x, omega, parent_idx, n_out):
    """Host prep. Returns (in_maps, meta)."""
    x = np.asarray(x)
    omega = np.asarray(omega, dtype=np.float32)
    parent = np.asarray(parent_idx).astype(np.int64)
    n_out_i = int(n_out)
    Bx, N, Cx = x.shape

    denom = np.bincount(parent, weights=omega.astype(np.float64),
                        minlength=n_out_i).astype(np.float32)
    wq = omega / np.maximum(denom, EPS)[parent]          # [N] f32

    perm = np.argsort(parent, kind="stable")
    sp_sorted = parent[perm]

    r = N // 2
    while 0 < r < N and sp_sorted[r - 1] == sp_sorted[r]:
        r += 1
    halves = [(0, r), (r, N)]

    scheds = []
    for lo, hi in halves:
        starts, ends, s_arr = _build_schedule(sp_sorted[lo:hi])
        scheds.append((lo, hi, starts, ends, s_arr))

    n_slots = max(len(s[2]) for s in scheds)
    n_slots = -(-n_slots // SPW) * SPW

    iota = np.broadcast_to(np.tile(np.arange(SLOT_W, dtype=np.float32), XB * SPW),
                           (P, XB * WIN_W))
    iota_bf = np.ascontiguousarray(iota.astype(ml_dtypes.bfloat16))

    half_data = []
    for (lo, hi, starts, ends, s_arr) in scheds:
        ns_real = len(starts)
        sps = sp_sorted[lo:hi]
        wqs = wq[perm[lo:hi]]
        srcrow = np.full((n_slots, P), -1, dtype=np.int64)
        PL = np.full((n_slots, P), -1.0, dtype=np.float32)
        WV = np.zeros((n_slots, P), dtype=np.float32)
        for t in range(ns_real):
            i0, i1, s = int(starts[t]), int(ends[t]), int(s_arr[t])
            n = i1 - i0
            srcrow[t, :n] = np.arange(i0, i1)
            PL[t, :n] = sps[i0:i1] - s
            WV[t, :n] = wqs[i0:i1]
        s_pad = np.zeros(n_slots, dtype=np.int64)
        s_pad[:ns_real] = s_arr
        if ns_real:
            s_pad[ns_real:] = s_arr[-1]
        orig = np.where(srcrow >= 0,
                        perm[lo:hi][np.clip(srcrow, 0, hi - lo - 1)], 0)
        half_data.append({
            "pl": np.ascontiguousarray(PL.T.astype(ml_dtypes.bfloat16)),
            "wv": np.ascontiguousarray(WV.T.astype(ml_dtypes.bfloat16)),
            "orig_rows": orig.reshape(-1),
            "s": s_pad,
            "ns_real": ns_real,
            "pbase": int(sp_sorted[lo]) if hi > lo else 0,
        })

    in_maps = []
    core_meta = []
    for b in range(Bx):
        for h in range(2):
            hd = half_data[h]
            xs = np.ascontiguousarray(
                x[b][hd["orig_rows"]].reshape(-1, P, Cx).transpose(1, 0, 2))
            in_maps.append({"xs": xs, "iota": iota_bf,
                            "pl": hd["pl"], "wv": hd["wv"]})
            core_meta.append((b, h))

    meta = {
        "n_slots": n_slots,
        "half_data": half_data,
        "core_meta": core_meta,
        "n_out": n_out_i,
        "B": Bx, "C": Cx,
    }
    return in_maps, meta


def stitch(results, meta):
    """results per core: {"out": [n_win, 128, WIN_W]} -> full output.

    Window layout (W-stationary orientation): slot j of a window sits at
    psum partitions [32*(j%4), +32) and free columns [C*(j//4), +C) --
    a [32 parents, C channels] block, parent-major.
    """
    n_out_i = meta["n_out"]
    out = np.zeros((meta["B"], n_out_i, meta["C"]), dtype=np.float32)
    Cx = meta["C"]
    for k, (b, h) in enumerate(meta["core_meta"]):
        hd = meta["half_data"][h]
        win = np.asarray(results[k]["out"]).astype(np.float32)
        pbase = hd["pbase"]
        width = n_out_i - pbase + SLOT_W
        buf = np.zeros((width, Cx), dtype=np.float32)
        s = hd["s"]
        for t in range(hd["ns_real"]):
            o = int(s[t]) - pbase
            j = t % SPW
            blk = win[t // SPW][32 * (j % 4):32 * (j % 4) + 32,
                               Cx * (j // 4):Cx * (j // 4) + Cx]
            buf[o:o + SLOT_W, :] += blk
        out[b, pbase:, :] += buf[:n_out_i - pbase, :]
    return out


def kernel(x, omega, parent_idx, n_out):
    global LAST_IN_MAPS, LAST_NC
    in_maps, meta = prep(x, omega, parent_idx, n_out)
    n_slots = meta["n_slots"]
    if n_slots not in _NC_CACHE:
        _NC_CACHE[n_slots] = build_nc(n_slots)
    nc = _NC_CACHE[n_slots]
    LAST_IN_MAPS, LAST_NC = in_maps, nc
    res = run_bass_kernel_spmd(nc, in_maps, core_ids=list(range(8)))
    return stitch(res.results, meta)



# revision 6
# speedup vs baseline: 3.7555x; 3.7555x over previous
"""AreaWeightedDownsample (segment reduce) for Trainium2, 8 NeuronCores.

out[b, p, c] = sum_{n: parent[n]==p} omega[n] * x[b,n,c] / max(sum omega[n], eps)

Strategy:
  Host: fold omega/denom into per-row weights w'; sort rows by parent; shard
  (4 batches) x (2 sorted-row halves) across 8 cores. Pack rows into "slots"
  of 128 rows whose parents span < SLOT_W consecutive values. Cast x to bf16
  on the host so the device reads half the bytes (DMA-bound kernel).
  Device (SPMD, identical instruction stream on all cores): for each window of
  slots: DMA rows of x (bf16); build the weighted-one-hot W tiles
  [128, SLOT_W] on VectorE from tiny per-slot tables
  (W[i, parent[i]-s_t] = w'[i] via iota==pl then *wv); SPW matmuls
  W^T @ x_slot -> psum quadrants; drain psum -> SBUF (bf16)
  -> DRAM. This computes out[parent, c] contributions per slot; the row
  reduction happens in the PE array.
  Host: overlap-add the SLOT_W-wide slot slices at their parent offsets.
"""

import os
import sys

for _p in ("/opt/trn_rl_repo", "/opt/pypackages"):
    if _p not in sys.path:
        sys.path.insert(0, _p)

import numpy as np
import ml_dtypes

from concourse import bacc, mybir
import concourse.tile as tile
import concourse.bass_utils as _bass_utils
from concourse.bass_utils import run_bass_kernel_spmd

if os.environ.get("ENABLE_LDW_OPT") and not getattr(_bass_utils, "_ldw_patched", False):
    _bass_utils._ldw_patched = True
    _orig_rc = _bass_utils.run_command

    def _rc(argv, **kw):
        argv = [a.replace("--enable-ldw-opt=false", "--enable-ldw-opt=true")
                if isinstance(a, str) else a for a in argv]
        return _orig_rc(argv, **kw)

    _bass_utils.run_command = _rc

B = 4
N_IN = 163842
C = 128
N_OUT = 40962
EPS = 1e-8

P = 128          # rows per slot
SLOT_W = 32      # psum columns per slot (max parent span within a slot)
SPW = 16         # slots per window (SLOT_W*SPW*4B = one 2KB psum bank)
WIN_W = SLOT_W * SPW  # psum columns per window (<= 512, one bank)
OB = 8           # windows per output DMA
XB = 2           # windows per x DMA

LAST_IN_MAPS = None
LAST_NC = None

_NC_CACHE = {}


def build_nc(n_slots, repeat=1, no_w=False, no_drain=False, no_mm=False):
    """Build the SPMD device graph for n_slots slots (multiple of 8).

    repeat > 1 replays the whole program (for timing); output is idempotent.
    """
    assert n_slots % SPW == 0
    n_win = n_slots // SPW

    nc = bacc.Bacc(None, target_bir_lowering=False)
    x_d = nc.dram_tensor("xs", [P, n_slots, C], mybir.dt.bfloat16,
                         kind="ExternalInput")
    iota_d = nc.dram_tensor("iota", [P, XB * WIN_W], mybir.dt.bfloat16,
                            kind="ExternalInput")
    pl_d = nc.dram_tensor("pl", [P, n_slots], mybir.dt.bfloat16,
                          kind="ExternalInput")
    wv_d = nc.dram_tensor("wv", [P, n_slots], mybir.dt.bfloat16,
                          kind="ExternalInput")
    o_d = nc.dram_tensor("out", [n_win, C, WIN_W], mybir.dt.bfloat16,
                         kind="ExternalOutput")

    with tile.TileContext(nc) as tc:
        with tc.tile_pool(name="cn", bufs=1) as cn, \
             tc.tile_pool(name="xp", bufs=3) as xp, \
             tc.tile_pool(name="wp", bufs=3) as wp, \
             tc.tile_pool(name="ip", bufs=3) as ip, \
             tc.tile_pool(name="sp", bufs=4) as sp, \
             tc.tile_pool(name="pp", bufs=6, space="PSUM") as pp:
            it = cn.tile([P, XB * WIN_W], mybir.dt.bfloat16)
            nc.sync.dma_start(out=it[:], in_=iota_d[:, :])
            plt = cn.tile([P, n_slots], mybir.dt.bfloat16)
            nc.sync.dma_start(out=plt[:], in_=pl_d[:, :])
            wvt = cn.tile([P, n_slots], mybir.dt.bfloat16)
            nc.sync.dma_start(out=wvt[:], in_=wv_d[:, :])
            wconst = None
            if no_w:
                wconst = cn.tile([P, XB * SPW, SLOT_W], mybir.dt.bfloat16)
                nc.vector.memset(wconst[:].rearrange("p t k -> p (t k)"), 0.5)

            xbatch = 0
            for _r in range(repeat):
                for g0 in range(0, n_win, OB):
                    gsz = min(OB, n_win - g0)
                    st = sp.tile([P, gsz, WIN_W], mybir.dt.bfloat16, tag="st")
                    for x0 in range(g0, g0 + gsz, XB):
                        xsz = min(XB, g0 + gsz - x0)
                        ns = xsz * SPW  # slots in this x-batch
                        xt = xp.tile([P, ns, C], mybir.dt.bfloat16, tag="xt")
                        # alternate the two HWDGE rings (SP / ACT)
                        eng = nc.sync if xbatch % 2 == 0 else nc.scalar
                        eng.dma_start(
                            out=xt[:],
                            in_=x_d[:, x0 * SPW:(x0 + xsz) * SPW, :],
                        )
                        xbatch += 1
                        iseq = ip.tile([P, ns, SLOT_W], mybir.dt.bfloat16,
                                       tag="iseq")
                        if no_w:
                            wt = wconst
                        else:
                            nc.vector.tensor_tensor(
                            out=iseq[:],
                            in0=it[:, :ns * SLOT_W].rearrange(
                                "p (t k) -> p t k", k=SLOT_W),
                            in1=plt[:, x0 * SPW:(x0 + xsz) * SPW][:, :, None]
                                .to_broadcast([P, ns, SLOT_W]),
                            op=mybir.AluOpType.is_equal,
                            )
                            wt = wp.tile([P, ns, SLOT_W], mybir.dt.bfloat16,
                                         tag="wt")
                            nc.vector.tensor_tensor(
                            out=wt[:],
                            in0=iseq[:],
                            in1=wvt[:, x0 * SPW:(x0 + xsz) * SPW][:, :, None]
                                .to_broadcast([P, ns, SLOT_W]),
                            op=mybir.AluOpType.mult,
                            )
                        for dw in range(xsz):
                            w = x0 + dw
                            pt = pp.tile([P, WIN_W], mybir.dt.float32)
                            for j in range(SPW if not no_mm else 1):
                                cg = j % 4          # psum col-group
                                fs = j // 4         # psum free-slot
                                nc.tensor.matmul(
                                    out=pt[32 * cg:32 * cg + 32,
                                           C * fs:C * fs + C],
                                    lhsT=wt[:, dw * SPW + j, :],
                                    rhs=xt[:, dw * SPW + j, :],
                                    start=True, stop=True,
                                    tile_position=(0, 32 * cg),
                                )
                            if no_drain:
                                continue
                            if w % 2 == 0:
                                nc.vector.tensor_copy(
                                    out=st[:, w - g0, :], in_=pt[:])
                            else:
                                nc.scalar.copy(
                                    out=st[:, w - g0, :], in_=pt[:])
                    if not no_drain:
                        nc.sync.dma_start(
                            out=o_d[g0:g0 + gsz].rearrange("g p k -> p g k"),
                            in_=st[:],
                        )
    nc.compile()
    return nc


def _build_schedule(sps):
    """Greedy slot packing of sorted parents sps (ascending, int).

    Returns (starts, ends, s_arr): slot t covers sorted-row range
    [starts[t], ends[t]) (<=128 rows), parents in [s_arr[t], s_arr[t]+SLOT_W).
    """
    R = len(sps)
    starts, ends, s_arr = [], [], []
    i = 0
    while i < R:
        s = int(sps[i])
        j = min(i + P, R)
        j = min(j, int(np.searchsorted(sps, s + SLOT_W, side="left")))
        starts.append(i)
        ends.append(j)
        s_arr.append(s)
        i = j
    return np.array(starts), np.array(ends), np.array(s_arr)


def prep(x, omega, parent_idx, n_out):
    """Host prep. Returns (in_maps, meta)."""
    x = np.asarray(x)
    omega = np.asarray(omega, dtype=np.float32)
    parent = np.asarray(parent_idx).astype(np.int64)
    n_out_i = int(n_out)
    Bx, N, Cx = x.shape

    denom = np.bincount(parent, weights=omega.astype(np.float64),
                        minlength=n_out_i).astype(np.float32)
    wq = omega / np.maximum(denom, EPS)[parent]          # [N] f32

    perm = np.argsort(parent, kind="stable")
    sp_sorted = parent[perm]

    r = N // 2
    while 0 < r < N and sp_sorted[r - 1] == sp_sorted[r]:
        r += 1
    halves = [(0, r), (r, N)]

    scheds = []
    for lo, hi in halves:
        starts, ends, s_arr = _build_schedule(sp_sorted[lo:hi])
        scheds.append((lo, hi, starts, ends, s_arr))

    n_slots = max(len(s[2]) for s in scheds)
    n_slots = -(-n_slots // SPW) * SPW

    iota = np.broadcast_to(np.tile(np.arange(SLOT_W, dtype=np.float32), XB * SPW),
                           (P, XB * WIN_W))
    iota_bf = np.ascontiguousarray(iota.astype(ml_dtypes.bfloat16))

    half_data = []
    for (lo, hi, starts, ends, s_arr) in scheds:
        ns_real = len(starts)
        sps = sp_sorted[lo:hi]
        wqs = wq[perm[lo:hi]]
        srcrow = np.full((n_slots, P), -1, dtype=np.int64)
        PL = np.full((n_slots, P), -1.0, dtype=np.float32)
        WV = np.zeros((n_slots, P), dtype=np.float32)
        for t in range(ns_real):
            i0, i1, s = int(starts[t]), int(ends[t]), int(s_arr[t])
            n = i1 - i0
            srcrow[t, :n] = np.arange(i0, i1)
            PL[t, :n] = sps[i0:i1] - s
            WV[t, :n] = wqs[i0:i1]
        s_pad = np.zeros(n_slots, dtype=np.int64)
        s_pad[:ns_real] = s_arr
        if ns_real:
            s_pad[ns_real:] = s_arr[-1]
        orig = np.where(srcrow >= 0,
                        perm[lo:hi][np.clip(srcrow, 0, hi - lo - 1)], 0)
        half_data.append({
            "pl": np.ascontiguousarray(PL.T.astype(ml_dtypes.bfloat16)),
            "wv": np.ascontiguousarray(WV.T.astype(ml_dtypes.bfloat16)),
            "orig_rows": orig.reshape(-1),
            "s": s_pad,
            "ns_real": ns_real,
            "pbase": int(sp_sorted[lo]) if hi > lo else 0,
        })

    xb16 = x.astype(ml_dtypes.bfloat16)   # host cast: device reads half the bytes
    in_maps = []
    core_meta = []
    for b in range(Bx):
        for h in range(2):
            hd = half_data[h]
            xs = np.ascontiguousarray(
                xb16[b][hd["orig_rows"]].reshape(-1, P, Cx).transpose(1, 0, 2))
            in_maps.append({"xs": xs, "iota": iota_bf,
                            "pl": hd["pl"], "wv": hd["wv"]})
            core_meta.append((b, h))

    meta = {
        "n_slots": n_slots,
        "half_data": half_data,
        "core_meta": core_meta,
        "n_out": n_out_i,
        "B": Bx, "C": Cx,
    }
    return in_maps, meta


def stitch(results, meta):
    """results per core: {"out": [n_win, 128, WIN_W]} -> full output.

    Window layout (W-stationary orientation): slot j of a window sits at
    psum partitions [32*(j%4), +32) and free columns [C*(j//4), +C) --
    a [32 parents, C channels] block, parent-major.
    """
    n_out_i = meta["n_out"]
    out = np.zeros((meta["B"], n_out_i, meta["C"]), dtype=np.float32)
    Cx = meta["C"]
    for k, (b, h) in enumerate(meta["core_meta"]):
        hd = meta["half_data"][h]
        win = np.asarray(results[k]["out"]).astype(np.float32)
        pbase = hd["pbase"]
        width = n_out_i - pbase + SLOT_W
        buf = np.zeros((width, Cx), dtype=np.float32)
        s = hd["s"]
        for t in range(hd["ns_real"]):
            o = int(s[t]) - pbase
            j = t % SPW
            blk = win[t // SPW][32 * (j % 4):32 * (j % 4) + 32,
                               Cx * (j // 4):Cx * (j // 4) + Cx]
            buf[o:o + SLOT_W, :] += blk
        out[b, pbase:, :] += buf[:n_out_i - pbase, :]
    return out


def kernel(x, omega, parent_idx, n_out):
    global LAST_IN_MAPS, LAST_NC
    in_maps, meta = prep(x, omega, parent_idx, n_out)
    n_slots = meta["n_slots"]
    if n_slots not in _NC_CACHE:
        _NC_CACHE[n_slots] = build_nc(n_slots)
    nc = _NC_CACHE[n_slots]
    LAST_IN_MAPS, LAST_NC = in_maps, nc
    res = run_bass_kernel_spmd(nc, in_maps, core_ids=list(range(8)))
    return stitch(res.results, meta)

